# revision 24
# baseline (speedup 1.0000x reference)
"""Trainium2 Bass kernel for nn_Dipole (gated equivariant MLP + segment reduce).

Strategy:
  - Shard 200000 atoms across 8 NeuronCores (overlapping 25088-atom windows,
    25088 = 49 * 512 so every core runs an identical fully-unrolled program).
  - On device (per core): cast-DMA l0/l1 to bf16, PE-transpose input tiles to
    feature-major layout, run both gated blocks as bf16 matmuls (weights
    replicated), emit per-atom dipole v [3] + charge s [1].
  - On host: y_atom = v + pos*charge, segment-sum per molecule via bincount.

Layouts are c-major over the full 512-atom block so every elementwise op is a
unit-stride [128, 512] op. vW matmuls are deferred until the gate is ready so
the gating multiply consumes vW directly from PSUM (no copy).

Self-contained: hardcodes shapes; only needs numpy + the concourse/axon stack
that ships with the container.
"""

import os
import sys

import numpy as np

for _p in ("/opt/trn_rl_repo", "/root/.axon_site/_ro/trn_rl_repo"):
    if os.path.isdir(_p) and _p not in sys.path:
        sys.path.append(_p)

import concourse.bacc as bacc
import concourse.bass as bass
import concourse.tile as tile
from concourse import mybir
from concourse.bass_utils import run_bass_kernel_spmd
from concourse.masks import make_identity

# run_bass_kernel_spmd(trace=True) imports antenv.axon_hooks, which this image
# lacks; register a working (or None) hook implementation up front so tracing
# degrades gracefully instead of crashing.
try:
    import antenv  # noqa
    if "antenv.axon_hooks" not in sys.modules:
        import types as _types

        def _mk_hook():
            try:
                from trn_agent_boot.trn_boot import _ntff_profile_via_ctypes
                return _ntff_profile_via_ctypes("/opt/axon/libaxon_pjrt.so")
            except Exception:
                return None

        _hook = _mk_hook()
        _mod = _types.ModuleType("antenv.axon_hooks")
        _mod.get_axon_ntff_profile_hook = lambda: _hook
        _mod.set_axon_ntff_profile_hook = lambda h: None
        sys.modules["antenv.axon_hooks"] = _mod
        antenv.axon_hooks = _mod
    import concourse.bass_utils as _bu

    def _local_upload(tmpdir):
        return str(tmpdir)

    _bu.upload_artifacts = _local_upload
except Exception:
    pass

F32 = mybir.dt.float32
BF16 = mybir.dt.bfloat16
AF = mybir.ActivationFunctionType
SILU = AF.Silu  # simcheck swaps to Tanh (CoreSim lacks Silu)

N_CORES = 8
N_ATOMS = 200000
N_IN = 256
N_HID = 256
N_MOL = 2048
BLK = 512                      # atoms per pipeline block
NQ = BLK // 128                # 128-atom quarters per block
SHARD = 25088                  # = 49 * BLK, per-core window
N_BLOCKS = SHARD // BLK
ATOMS_PER_CORE = N_ATOMS // N_CORES  # 25000 rows of output actually used


def bcast_c(ap2d, c=3):
    """[P, A] AP -> [P, c, A] with a step-0 broadcast middle dim."""
    return bass.AP(tensor=ap2d.tensor, offset=ap2d.offset,
                   ap=[ap2d.ap[0], [0, c], ap2d.ap[1]])


def build_nc(n_blocks=N_BLOCKS):
    n = n_blocks * BLK
    nc = bacc.Bacc("TRN2", target_bir_lowering=False, debug=False)

    l0 = nc.dram_tensor("l0", [n, N_IN], F32, kind="ExternalInput")
    l1 = nc.dram_tensor("l1", [n, 3, N_IN], F32, kind="ExternalInput")
    Wmix1 = nc.dram_tensor("Wmix1", [N_IN, 2 * N_HID], F32, kind="ExternalInput")
    W1a = nc.dram_tensor("W1a", [N_IN + N_HID, N_HID], F32, kind="ExternalInput")
    b1a = nc.dram_tensor("b1a", [N_HID], F32, kind="ExternalInput")
    W2a = nc.dram_tensor("W2a", [N_HID, 2 * N_HID], F32, kind="ExternalInput")
    b2a = nc.dram_tensor("b2a", [2 * N_HID], F32, kind="ExternalInput")
    Wmix2 = nc.dram_tensor("Wmix2", [N_HID, 2], F32, kind="ExternalInput")
    W1b = nc.dram_tensor("W1b", [N_HID + 1, N_HID], F32, kind="ExternalInput")
    b1b = nc.dram_tensor("b1b", [N_HID], F32, kind="ExternalInput")
    W2b = nc.dram_tensor("W2b", [N_HID, 2], F32, kind="ExternalInput")
    b2b = nc.dram_tensor("b2b", [2], F32, kind="ExternalInput")
    out = nc.dram_tensor("out", [4, n], F32, kind="ExternalOutput")

    with tile.TileContext(nc) as tc:
        with (
            tc.tile_pool(name="singles", bufs=1) as singles,
            tc.tile_pool(name="pin", bufs=5) as pin,
            tc.tile_pool(name="ptr", bufs=2) as ptr,
            tc.tile_pool(name="pact", bufs=2) as pact,
            tc.tile_pool(name="pmisc", bufs=3) as pmisc,
            tc.tile_pool(name="pout", bufs=3) as pout,
            tc.tile_pool(name="ps_tr", bufs=2, space="PSUM") as ps_tr,
            tc.tile_pool(name="ps_a", bufs=2, space="PSUM") as ps_a,
            tc.tile_pool(name="ps_b", bufs=2, space="PSUM") as ps_b,
            tc.tile_pool(name="ps_c", bufs=2, space="PSUM") as ps_c,
        ):
            # ---- constants / weights (bf16 via SWDGE cast-DMA) ----
            ident = singles.tile([128, 128], BF16)
            make_identity(nc, ident)

            wmix1_sb = singles.tile([128, 2, 512], BF16)
            nc.gpsimd.dma_start(
                out=wmix1_sb, in_=Wmix1.rearrange("(ko ki) o -> ki ko o", ki=128))
            w1a_sb = singles.tile([128, 4, 256], BF16)
            nc.gpsimd.dma_start(
                out=w1a_sb, in_=W1a.rearrange("(ko ki) o -> ki ko o", ki=128))
            w2a_sb = singles.tile([128, 2, 512], BF16)
            nc.gpsimd.dma_start(
                out=w2a_sb, in_=W2a.rearrange("(ko ki) o -> ki ko o", ki=128))
            w1b_sb = singles.tile([128, 2, 256], BF16)
            nc.gpsimd.dma_start(
                out=w1b_sb, in_=W1b[0:256, :].rearrange("(ko ki) o -> ki ko o", ki=128))
            w1bL_sb = singles.tile([1, 256], BF16)
            nc.gpsimd.dma_start(out=w1bL_sb, in_=W1b[256:257, :])
            w2b_sb = singles.tile([128, 2, 2], BF16)
            nc.gpsimd.dma_start(
                out=w2b_sb, in_=W2b.rearrange("(ko ki) o -> ki ko o", ki=128))
            wmix2_sb = singles.tile([128, 2, 2], BF16)
            nc.gpsimd.dma_start(
                out=wmix2_sb, in_=Wmix2.rearrange("(ko ki) o -> ki ko o", ki=128))

            b1a_sb = singles.tile([128, 2], F32)
            nc.sync.dma_start(out=b1a_sb, in_=b1a.rearrange("(mo mi) -> mi mo", mi=128))
            b2a_sb = singles.tile([128, 4], F32)
            nc.sync.dma_start(out=b2a_sb, in_=b2a.rearrange("(mo mi) -> mi mo", mi=128))
            b1b_sb = singles.tile([128, 2], F32)
            nc.sync.dma_start(out=b1b_sb, in_=b1b.rearrange("(mo mi) -> mi mo", mi=128))
            b2b_sb = singles.tile([2, 1], F32)
            nc.sync.dma_start(out=b2b_sb, in_=b2b.rearrange("(p o) -> p o", o=1))

            for b in range(n_blocks):
                a0 = b * BLK
                # ---- load + cast inputs (atom-major) ----
                l0_t = pin.tile([128, NQ, 256], BF16, tag="l0_t")
                nc.gpsimd.dma_start(
                    out=l0_t,
                    in_=l0[a0:a0 + BLK, :].rearrange("(ao ai) f -> ai ao f", ai=128))
                l1_t = pin.tile([128, NQ, 3, 256], BF16, tag="l1_t")
                nc.gpsimd.dma_start(
                    out=l1_t,
                    in_=l1[a0:a0 + BLK, :, :].rearrange("(ao ai) c f -> ai ao c f", ai=128))

                # ---- transpose to feature-major (quarter layout) ----
                # l0T: (ki, kt, q*128+a); l1T: (ki, kt, q, (c, a))
                l0T = ptr.tile([128, 2, BLK], BF16, tag="l0T")
                l1T = ptr.tile([128, 2, NQ, 384], BF16, tag="l1T")
                for q in range(NQ):
                    for kt in range(2):
                        trp = ps_tr.tile([128, 512], BF16, tag="trp", name="trp")
                        for c in range(3):
                            nc.tensor.transpose(
                                trp[:, c * 128:(c + 1) * 128],
                                l1_t[:, q, c, kt * 128:(kt + 1) * 128], ident)
                        nc.tensor.transpose(
                            trp[:, 384:512], l0_t[:, q, kt * 128:(kt + 1) * 128], ident)
                        nc.vector.tensor_copy(l1T[:, kt, q, :], trp[:, 0:384])
                        nc.vector.tensor_copy(
                            l0T[:, kt, q * 128:(q + 1) * 128], trp[:, 384:512])

                # ---- block 1: vmix1; vVn; vW kept in SBUF ----
                vvnsq = pmisc.tile([128, 2, BLK], F32, tag="vvnsq")
                vvn_bf = pact.tile([128, 2, BLK], BF16, tag="vvn")
                vw_sb = pact.tile([128, 2, NQ, 384], BF16, tag="vw")
                for mt in range(4):
                    for qp in range(NQ // 2):
                        vps = [None, None]
                        for kt in range(2):          # weight reused across qi
                            for qi in range(2):
                                if kt == 0:
                                    vps[qi] = ps_a.tile([128, 384], F32, tag="vps", name="vps")
                                nc.tensor.matmul(
                                    vps[qi],
                                    wmix1_sb[:, kt, mt * 128:(mt + 1) * 128],
                                    l1T[:, kt, qp * 2 + qi, :],
                                    start=(kt == 0), stop=(kt == 1))
                        for qi in range(2):
                            q = qp * 2 + qi
                            if mt < 2:
                                sq = pmisc.tile([128, 384], F32, tag="sq", name="sq")
                                nc.scalar.square(sq, vps[qi])
                                vq = vvnsq[:, mt, q * 128:(q + 1) * 128]
                                nc.gpsimd.tensor_add(vq, sq[:, 0:128], sq[:, 128:256])
                                nc.gpsimd.tensor_add(vq, vq, sq[:, 256:384])
                            else:
                                nc.vector.tensor_copy(
                                    vw_sb[:, mt - 2, q, :], vps[qi])
                for mt in range(2):
                    nc.scalar.sqrt(vvn_bf[:, mt, :], vvnsq[:, mt, :])

                # ---- block 1 MLP ----
                h1_bf = pact.tile([128, 2, BLK], BF16, tag="h1")
                for mt in range(2):
                    hps = ps_b.tile([128, BLK], F32, tag="hps", name="hps")
                    for kt in range(4):
                        rhs = l0T[:, kt, :] if kt < 2 else vvn_bf[:, kt - 2, :]
                        nc.tensor.matmul(
                            hps, w1a_sb[:, kt, mt * 128:(mt + 1) * 128], rhs,
                            start=(kt == 0), stop=(kt == 3))
                    nc.scalar.activation(
                        out=h1_bf[:, mt, :], in_=hps, func=SILU,
                        bias=b1a_sb[:, mt:mt + 1], scale=1.0)

                s_bf = pact.tile([128, 2, BLK], BF16, tag="s")
                gate_bf = pact.tile([128, 2, BLK], BF16, tag="gate")
                for mt in range(4):
                    xps = ps_b.tile([128, BLK], F32, tag="hps", name="xps")
                    for kt in range(2):
                        nc.tensor.matmul(
                            xps, w2a_sb[:, kt, mt * 128:(mt + 1) * 128], h1_bf[:, kt, :],
                            start=(kt == 0), stop=(kt == 1))
                    if mt < 2:
                        nc.scalar.activation(
                            out=s_bf[:, mt, :], in_=xps, func=SILU,
                            bias=b2a_sb[:, mt:mt + 1], scale=1.0)
                    else:
                        nc.vector.tensor_scalar_add(
                            out=gate_bf[:, mt - 2, :], in0=xps,
                            scalar1=b2a_sb[:, mt:mt + 1])

                # ---- gating: v_out = gate * vW (GPSIMD, SBUF only) ----
                vout_bf = pact.tile([128, 2, NQ, 384], BF16, tag="vout")
                for mt in range(2):
                    for q in range(NQ):
                        nc.vector.tensor_mul(
                            vout_bf[:, mt, q, :], vw_sb[:, mt, q, :],
                            bcast_c(gate_bf[:, mt, q * 128:(q + 1) * 128]))

                # ---- block 2: vmix2 (M=2 per c at 32-aligned strips) ----
                smp = ps_c.tile([128, BLK], F32, tag="csm", name="smp")
                for kt in range(2):                 # weight reused across c
                    for c in range(3):
                        nc.tensor.matmul(
                            smp[32 * c:32 * c + 2, :],
                            wmix2_sb[:, kt, :],
                            vout_bf[:, kt, :, c * 128:(c + 1) * 128],
                            start=(kt == 0), stop=(kt == 1))

                # vVn2 = sqrt(sum_c vV2^2); vV2 = rows 32c, vW2 = rows 32c+1.
                t_sq = pmisc.tile([2, 3, BLK], F32, tag="t_sq")
                for c in range(3):
                    nc.scalar.square(t_sq[:, c, :], smp[32 * c:32 * c + 2, :])
                vvn2sq = pmisc.tile([1, BLK], F32, tag="vvn2sq")
                nc.gpsimd.tensor_add(vvn2sq, t_sq[0:1, 0, :], t_sq[0:1, 1, :])
                nc.gpsimd.tensor_add(vvn2sq, vvn2sq, t_sq[0:1, 2, :])
                vvn2_bf = pmisc.tile([1, BLK], BF16, tag="vvn2bf")
                nc.scalar.sqrt(vvn2_bf, vvn2sq)

                # ---- block 2 MLP ----
                h2_bf = pact.tile([128, 2, BLK], BF16, tag="h2")
                for mt in range(2):
                    h2ps = ps_b.tile([128, BLK], F32, tag="hps", name="h2ps")
                    for kt in range(2):
                        nc.tensor.matmul(
                            h2ps, w1b_sb[:, kt, mt * 128:(mt + 1) * 128], s_bf[:, kt, :],
                            start=(kt == 0), stop=False)
                    nc.tensor.matmul(
                        h2ps, w1bL_sb[0:1, mt * 128:(mt + 1) * 128], vvn2_bf,
                        start=False, stop=True)
                    nc.scalar.activation(
                        out=h2_bf[:, mt, :], in_=h2ps, func=SILU,
                        bias=b1b_sb[:, mt:mt + 1], scale=1.0)

                x3ps = ps_c.tile([2, BLK], F32, tag="csm", name="x3ps")
                for kt in range(2):
                    nc.tensor.matmul(
                        x3ps, w2b_sb[:, kt, :], h2_bf[:, kt, :],
                        start=(kt == 0), stop=(kt == 1))
                x3_sb = pout.tile([2, BLK], F32, tag="x3sb")
                nc.vector.tensor_scalar_add(out=x3_sb, in0=x3ps,
                                            scalar1=b2b_sb[0:2, 0:1])

                # ---- v_final = gate2 * vW2 (row 1 of each [2,.] pair) ----
                vfin = pout.tile([2, 3, BLK], F32, tag="vfin")
                for c in range(3):
                    nc.vector.tensor_mul(
                        vfin[:, c, :], smp[32 * c:32 * c + 2, :], x3_sb[0:2, :])

                nc.sync.dma_start(out=out[0:3, a0:a0 + BLK], in_=vfin[1:2, :, :])
                nc.sync.dma_start(out=out[3:4, a0:a0 + BLK], in_=x3_sb[0:1, :])

    nc.compile()
    return nc


_NC_CACHE = {}


def _get_nc(n_blocks=N_BLOCKS, finalized=True):
    key = (n_blocks, finalized)
    if key not in _NC_CACHE:
        nc = build_nc(n_blocks)
        if finalized:
            nc.finalize()
        _NC_CACHE[key] = nc
    return _NC_CACHE[key]


def kernel(pos, l0, l1, batch, Wmix1, W1a, b1a, W2a, b2a,
           Wmix2, W1b, b1b, W2b, b2b, num_segments, trace=False, tmpdir=None):
    pos = np.asarray(pos, dtype=np.float32)
    l0 = np.asarray(l0, dtype=np.float32)
    l1 = np.asarray(l1, dtype=np.float32)
    batch_np = np.asarray(batch).astype(np.int64)
    nseg = int(num_segments)

    nc = _get_nc()

    weights = dict(Wmix1=Wmix1, W1a=W1a, b1a=b1a, W2a=W2a, b2a=b2a,
                   Wmix2=Wmix2, W1b=W1b, b1b=b1b, W2b=W2b, b2b=b2b)
    weights = {k: np.ascontiguousarray(np.asarray(v, dtype=np.float32))
               for k, v in weights.items()}

    starts = [min(c * ATOMS_PER_CORE, N_ATOMS - SHARD) for c in range(N_CORES)]
    in_maps = []
    for c in range(N_CORES):
        s = starts[c]
        m = dict(weights)
        m["l0"] = l0[s:s + SHARD]
        m["l1"] = l1[s:s + SHARD]
        in_maps.append(m)

    res = run_bass_kernel_spmd(nc, in_maps, core_ids=list(range(N_CORES)),
                               trace=trace, tmpdir=tmpdir)

    # reassemble per-atom v (dipole) and s (charge)
    v = np.empty((N_ATOMS, 3), dtype=np.float32)
    charge = np.empty((N_ATOMS,), dtype=np.float32)
    for c in range(N_CORES):
        o = res.results[c]["out"]  # [4, SHARD]
        lo = c * ATOMS_PER_CORE
        off = lo - starts[c]
        v[lo:lo + ATOMS_PER_CORE] = o[0:3, off:off + ATOMS_PER_CORE].T
        charge[lo:lo + ATOMS_PER_CORE] = o[3, off:off + ATOMS_PER_CORE]

    y_atom = v + pos * charge[:, None]
    y = np.stack(
        [np.bincount(batch_np, weights=y_atom[:, k].astype(np.float64),
                     minlength=nseg)[:nseg] for k in range(3)],
        axis=1).astype(np.float32)
    y_vec = np.stack(
        [np.bincount(batch_np, weights=v[:, k].astype(np.float64),
                     minlength=nseg)[:nseg] for k in range(3)],
        axis=1).astype(np.float32)[..., None]

    if trace:
        kernel.last_results = res
    return y, y_vec


kernel.last_results = None


# revision 26
# speedup vs baseline: 1.3267x; 1.3267x over previous
"""Trainium2 Bass kernel for nn_Dipole (gated equivariant MLP + segment reduce).

Strategy:
  - Shard 200000 atoms across 8 NeuronCores (overlapping 25088-atom windows,
    25088 = 49 * 512 so every core runs an identical fully-unrolled program).
  - On device (per core): cast-DMA l0/l1 to bf16, PE-transpose input tiles to
    feature-major layout, run both gated blocks as bf16 matmuls (weights
    replicated), emit per-atom dipole v [3] + charge s [1].
  - On host: y_atom = v + pos*charge, segment-sum per molecule via bincount.

Layouts are c-major over the full 512-atom block so every elementwise op is a
unit-stride [128, 512] op. vW matmuls are deferred until the gate is ready so
the gating multiply consumes vW directly from PSUM (no copy).

Self-contained: hardcodes shapes; only needs numpy + the concourse/axon stack
that ships with the container.
"""

import os
import sys

import numpy as np

for _p in ("/opt/trn_rl_repo", "/root/.axon_site/_ro/trn_rl_repo"):
    if os.path.isdir(_p) and _p not in sys.path:
        sys.path.append(_p)

import concourse.bacc as bacc
import concourse.bass as bass
import concourse.tile as tile
from concourse import mybir
from concourse.bass_utils import run_bass_kernel_spmd
from concourse.masks import make_identity

# run_bass_kernel_spmd(trace=True) imports antenv.axon_hooks, which this image
# lacks; register a working (or None) hook implementation up front so tracing
# degrades gracefully instead of crashing.
try:
    import antenv  # noqa
    if "antenv.axon_hooks" not in sys.modules:
        import types as _types

        def _mk_hook():
            try:
                from trn_agent_boot.trn_boot import _ntff_profile_via_ctypes
                return _ntff_profile_via_ctypes("/opt/axon/libaxon_pjrt.so")
            except Exception:
                return None

        _hook = _mk_hook()
        _mod = _types.ModuleType("antenv.axon_hooks")
        _mod.get_axon_ntff_profile_hook = lambda: _hook
        _mod.set_axon_ntff_profile_hook = lambda h: None
        sys.modules["antenv.axon_hooks"] = _mod
        antenv.axon_hooks = _mod
    import concourse.bass_utils as _bu

    def _local_upload(tmpdir):
        return str(tmpdir)

    _bu.upload_artifacts = _local_upload
except Exception:
    pass

F32 = mybir.dt.float32
BF16 = mybir.dt.bfloat16
AF = mybir.ActivationFunctionType
SILU = AF.Silu  # simcheck swaps to Tanh (CoreSim lacks Silu)

N_CORES = 8
N_ATOMS = 200000
N_IN = 256
N_HID = 256
N_MOL = 2048
BLK = 512                      # atoms per pipeline block
NQ = BLK // 128                # 128-atom quarters per block
SHARD = 25088                  # = 49 * BLK, per-core window
N_BLOCKS = SHARD // BLK
ATOMS_PER_CORE = N_ATOMS // N_CORES  # 25000 rows of output actually used


def bcast_c(ap2d, c=3):
    """[P, A] AP -> [P, c, A] with a step-0 broadcast middle dim."""
    return bass.AP(tensor=ap2d.tensor, offset=ap2d.offset,
                   ap=[ap2d.ap[0], [0, c], ap2d.ap[1]])


def build_nc(n_blocks=N_BLOCKS):
    n = n_blocks * BLK
    nc = bacc.Bacc("TRN2", target_bir_lowering=False, debug=False)

    l0 = nc.dram_tensor("l0", [n, N_IN], F32, kind="ExternalInput")
    l1 = nc.dram_tensor("l1", [n, 3, N_IN], F32, kind="ExternalInput")
    Wmix1 = nc.dram_tensor("Wmix1", [N_IN, 2 * N_HID], F32, kind="ExternalInput")
    W1a = nc.dram_tensor("W1a", [N_IN + N_HID, N_HID], F32, kind="ExternalInput")
    b1a = nc.dram_tensor("b1a", [N_HID], F32, kind="ExternalInput")
    W2a = nc.dram_tensor("W2a", [N_HID, 2 * N_HID], F32, kind="ExternalInput")
    b2a = nc.dram_tensor("b2a", [2 * N_HID], F32, kind="ExternalInput")
    Wmix2 = nc.dram_tensor("Wmix2", [N_HID, 2], F32, kind="ExternalInput")
    W1b = nc.dram_tensor("W1b", [N_HID + 1, N_HID], F32, kind="ExternalInput")
    b1b = nc.dram_tensor("b1b", [N_HID], F32, kind="ExternalInput")
    W2b = nc.dram_tensor("W2b", [N_HID, 2], F32, kind="ExternalInput")
    b2b = nc.dram_tensor("b2b", [2], F32, kind="ExternalInput")
    out = nc.dram_tensor("out", [4, n], F32, kind="ExternalOutput")

    with tile.TileContext(nc) as tc:
        with (
            tc.tile_pool(name="singles", bufs=1) as singles,
            tc.tile_pool(name="pin", bufs=5) as pin,
            tc.tile_pool(name="ptr", bufs=2) as ptr,
            tc.tile_pool(name="pact", bufs=2) as pact,
            tc.tile_pool(name="pmisc", bufs=3) as pmisc,
            tc.tile_pool(name="pout", bufs=3) as pout,
            tc.tile_pool(name="ps_tr", bufs=2, space="PSUM") as ps_tr,
            tc.tile_pool(name="ps_a", bufs=2, space="PSUM") as ps_a,
            tc.tile_pool(name="ps_b", bufs=2, space="PSUM") as ps_b,
            tc.tile_pool(name="ps_c", bufs=2, space="PSUM") as ps_c,
        ):
            # ---- constants / weights (bf16 via SWDGE cast-DMA) ----
            ident = singles.tile([128, 128], BF16)
            make_identity(nc, ident)

            wmix1_sb = singles.tile([128, 2, 512], BF16)
            nc.gpsimd.dma_start(
                out=wmix1_sb, in_=Wmix1.rearrange("(ko ki) o -> ki ko o", ki=128))
            w1a_sb = singles.tile([128, 4, 256], BF16)
            nc.gpsimd.dma_start(
                out=w1a_sb, in_=W1a.rearrange("(ko ki) o -> ki ko o", ki=128))
            w2a_sb = singles.tile([128, 2, 512], BF16)
            nc.gpsimd.dma_start(
                out=w2a_sb, in_=W2a.rearrange("(ko ki) o -> ki ko o", ki=128))
            w1b_sb = singles.tile([128, 2, 256], BF16)
            nc.gpsimd.dma_start(
                out=w1b_sb, in_=W1b[0:256, :].rearrange("(ko ki) o -> ki ko o", ki=128))
            w1bL_sb = singles.tile([1, 256], BF16)
            nc.gpsimd.dma_start(out=w1bL_sb, in_=W1b[256:257, :])
            w2b_sb = singles.tile([128, 2, 2], BF16)
            nc.gpsimd.dma_start(
                out=w2b_sb, in_=W2b.rearrange("(ko ki) o -> ki ko o", ki=128))
            wmix2_sb = singles.tile([128, 2, 2], BF16)
            nc.gpsimd.dma_start(
                out=wmix2_sb, in_=Wmix2.rearrange("(ko ki) o -> ki ko o", ki=128))

            b1a_sb = singles.tile([128, 2], F32)
            nc.sync.dma_start(out=b1a_sb, in_=b1a.rearrange("(mo mi) -> mi mo", mi=128))
            b2a_sb = singles.tile([128, 4], F32)
            nc.sync.dma_start(out=b2a_sb, in_=b2a.rearrange("(mo mi) -> mi mo", mi=128))
            b1b_sb = singles.tile([128, 2], F32)
            nc.sync.dma_start(out=b1b_sb, in_=b1b.rearrange("(mo mi) -> mi mo", mi=128))
            b2b_sb = singles.tile([2, 1], F32)
            nc.sync.dma_start(out=b2b_sb, in_=b2b.rearrange("(p o) -> p o", o=1))

            for b in range(n_blocks):
                a0 = b * BLK
                # ---- load + cast inputs (atom-major) ----
                l0_t = pin.tile([128, NQ, 256], BF16, tag="l0_t")
                nc.gpsimd.dma_start(
                    out=l0_t,
                    in_=l0[a0:a0 + BLK, :].rearrange("(ao ai) f -> ai ao f", ai=128))
                l1_t = pin.tile([128, NQ, 3, 256], BF16, tag="l1_t")
                nc.gpsimd.dma_start(
                    out=l1_t,
                    in_=l1[a0:a0 + BLK, :, :].rearrange("(ao ai) c f -> ai ao c f", ai=128))

                # ---- transpose to feature-major (quarter layout) ----
                # l0T: (ki, kt, q*128+a); l1T: (ki, kt, q, (c, a))
                l0T = ptr.tile([128, 2, BLK], BF16, tag="l0T")
                l1T = ptr.tile([128, 2, NQ, 384], BF16, tag="l1T")
                for qp in range(NQ // 2):
                    for kt in range(2):
                        trp = ps_tr.tile([128, 1024], BF16, tag="trp", name="trp")
                        for qi in range(2):
                            q = qp * 2 + qi
                            for c in range(3):
                                nc.tensor.transpose(
                                    trp[:, qi * 384 + c * 128:qi * 384 + (c + 1) * 128],
                                    l1_t[:, q, c, kt * 128:(kt + 1) * 128], ident)
                            nc.tensor.transpose(
                                trp[:, 768 + q % 2 * 128:768 + (q % 2 + 1) * 128],
                                l0_t[:, q, kt * 128:(kt + 1) * 128], ident)
                        nc.vector.tensor_copy(
                            l1T[:, kt, qp * 2:qp * 2 + 2, :], trp[:, 0:768])
                        nc.scalar.copy(
                            l0T[:, kt, qp * 256:(qp + 1) * 256], trp[:, 768:1024])

                # ---- block 1: vmix1; vVn; vW kept in SBUF ----
                vvnsq = pmisc.tile([128, 2, BLK], F32, tag="vvnsq")
                vvn_bf = pact.tile([128, 2, BLK], BF16, tag="vvn")
                vw_sb = pact.tile([128, 2, NQ, 384], BF16, tag="vw")
                for mt in range(4):
                    for qp in range(NQ // 2):
                        vps = [None, None]
                        for kt in range(2):          # weight reused across qi
                            for qi in range(2):
                                if kt == 0:
                                    vps[qi] = ps_a.tile([128, 384], F32, tag="vps", name="vps")
                                nc.tensor.matmul(
                                    vps[qi],
                                    wmix1_sb[:, kt, mt * 128:(mt + 1) * 128],
                                    l1T[:, kt, qp * 2 + qi, :],
                                    start=(kt == 0), stop=(kt == 1))
                        for qi in range(2):
                            q = qp * 2 + qi
                            if mt < 2:
                                sq = pmisc.tile([128, 384], F32, tag="sq", name="sq")
                                nc.scalar.square(sq, vps[qi])
                                nc.vector.tensor_reduce(
                                    vvnsq[:, mt, q * 128:(q + 1) * 128],
                                    sq[:, :].rearrange("p (c a) -> p a c", c=3),
                                    axis=mybir.AxisListType.X, op=mybir.AluOpType.add)
                            else:
                                nc.vector.tensor_copy(
                                    vw_sb[:, mt - 2, q, :], vps[qi])
                nc.scalar.sqrt(vvn_bf[:, :, :], vvnsq[:, :, :])

                # ---- block 1 MLP ----
                h1_bf = pact.tile([128, 2, BLK], BF16, tag="h1")
                for mt in range(2):
                    hps = ps_b.tile([128, BLK], F32, tag="hps", name="hps")
                    for kt in range(4):
                        rhs = l0T[:, kt, :] if kt < 2 else vvn_bf[:, kt - 2, :]
                        nc.tensor.matmul(
                            hps, w1a_sb[:, kt, mt * 128:(mt + 1) * 128], rhs,
                            start=(kt == 0), stop=(kt == 3))
                    nc.scalar.activation(
                        out=h1_bf[:, mt, :], in_=hps, func=SILU,
                        bias=b1a_sb[:, mt:mt + 1], scale=1.0)

                s_bf = pact.tile([128, 2, BLK], BF16, tag="s")
                gate_bf = pact.tile([128, 2, BLK], BF16, tag="gate")
                for mt in range(4):
                    xps = ps_b.tile([128, BLK], F32, tag="hps", name="xps")
                    for kt in range(2):
                        nc.tensor.matmul(
                            xps, w2a_sb[:, kt, mt * 128:(mt + 1) * 128], h1_bf[:, kt, :],
                            start=(kt == 0), stop=(kt == 1))
                    if mt < 2:
                        nc.scalar.activation(
                            out=s_bf[:, mt, :], in_=xps, func=SILU,
                            bias=b2a_sb[:, mt:mt + 1], scale=1.0)
                    else:
                        nc.scalar.activation(
                            out=gate_bf[:, mt - 2, :], in_=xps, func=AF.Identity,
                            bias=b2a_sb[:, mt:mt + 1], scale=1.0)

                # ---- gating: v_out = gate * vW (GPSIMD, SBUF only) ----
                vout_bf = pact.tile([128, 2, NQ, 384], BF16, tag="vout")
                for mt in range(2):
                    for q in range(NQ):
                        nc.vector.tensor_mul(
                            vout_bf[:, mt, q, :], vw_sb[:, mt, q, :],
                            bcast_c(gate_bf[:, mt, q * 128:(q + 1) * 128]))

                # ---- block 2: vmix2 (M=2 per c at 32-aligned strips) ----
                smp = ps_c.tile([128, BLK], F32, tag="csm", name="smp")
                for kt in range(2):                 # weight reused across c
                    for c in range(3):
                        nc.tensor.matmul(
                            smp[32 * c:32 * c + 2, :],
                            wmix2_sb[:, kt, :],
                            vout_bf[:, kt, :, c * 128:(c + 1) * 128],
                            start=(kt == 0), stop=(kt == 1))

                # vVn2 = sqrt(sum_c vV2^2); vV2 = rows 32c, vW2 = rows 32c+1.
                t_sq = pmisc.tile([2, 3, BLK], F32, tag="t_sq")
                for c in range(3):
                    nc.scalar.square(t_sq[:, c, :], smp[32 * c:32 * c + 2, :])
                vvn2sq = pmisc.tile([1, BLK], F32, tag="vvn2sq")
                nc.gpsimd.tensor_add(vvn2sq, t_sq[0:1, 0, :], t_sq[0:1, 1, :])
                nc.gpsimd.tensor_add(vvn2sq, vvn2sq, t_sq[0:1, 2, :])
                vvn2_bf = pmisc.tile([1, BLK], BF16, tag="vvn2bf")
                nc.scalar.sqrt(vvn2_bf, vvn2sq)

                # ---- block 2 MLP ----
                h2_bf = pact.tile([128, 2, BLK], BF16, tag="h2")
                for mt in range(2):
                    h2ps = ps_b.tile([128, BLK], F32, tag="hps", name="h2ps")
                    for kt in range(2):
                        nc.tensor.matmul(
                            h2ps, w1b_sb[:, kt, mt * 128:(mt + 1) * 128], s_bf[:, kt, :],
                            start=(kt == 0), stop=False)
                    nc.tensor.matmul(
                        h2ps, w1bL_sb[0:1, mt * 128:(mt + 1) * 128], vvn2_bf,
                        start=False, stop=True)
                    nc.scalar.activation(
                        out=h2_bf[:, mt, :], in_=h2ps, func=SILU,
                        bias=b1b_sb[:, mt:mt + 1], scale=1.0)

                x3ps = ps_c.tile([2, BLK], F32, tag="csm", name="x3ps")
                for kt in range(2):
                    nc.tensor.matmul(
                        x3ps, w2b_sb[:, kt, :], h2_bf[:, kt, :],
                        start=(kt == 0), stop=(kt == 1))
                x3_sb = pout.tile([2, BLK], F32, tag="x3sb")
                nc.scalar.activation(
                    out=x3_sb, in_=x3ps, func=AF.Identity,
                    bias=b2b_sb[0:2, 0:1], scale=1.0)

                # ---- v_final = gate2 * vW2 (row 1 of each [2,.] pair) ----
                vfin = pout.tile([2, 3, BLK], F32, tag="vfin")
                for c in range(3):
                    nc.vector.tensor_mul(
                        vfin[:, c, :], smp[32 * c:32 * c + 2, :], x3_sb[0:2, :])

                nc.sync.dma_start(out=out[0:3, a0:a0 + BLK], in_=vfin[1:2, :, :])
                nc.sync.dma_start(out=out[3:4, a0:a0 + BLK], in_=x3_sb[0:1, :])

    nc.compile()
    return nc


_NC_CACHE = {}


def _get_nc(n_blocks=N_BLOCKS, finalized=True):
    key = (n_blocks, finalized)
    if key not in _NC_CACHE:
        nc = build_nc(n_blocks)
        if finalized:
            nc.finalize()
        _NC_CACHE[key] = nc
    return _NC_CACHE[key]


def kernel(pos, l0, l1, batch, Wmix1, W1a, b1a, W2a, b2a,
           Wmix2, W1b, b1b, W2b, b2b, num_segments, trace=False, tmpdir=None):
    pos = np.asarray(pos, dtype=np.float32)
    l0 = np.asarray(l0, dtype=np.float32)
    l1 = np.asarray(l1, dtype=np.float32)
    batch_np = np.asarray(batch).astype(np.int64)
    nseg = int(num_segments)

    nc = _get_nc()

    weights = dict(Wmix1=Wmix1, W1a=W1a, b1a=b1a, W2a=W2a, b2a=b2a,
                   Wmix2=Wmix2, W1b=W1b, b1b=b1b, W2b=W2b, b2b=b2b)
    weights = {k: np.ascontiguousarray(np.asarray(v, dtype=np.float32))
               for k, v in weights.items()}

    starts = [min(c * ATOMS_PER_CORE, N_ATOMS - SHARD) for c in range(N_CORES)]
    in_maps = []
    for c in range(N_CORES):
        s = starts[c]
        m = dict(weights)
        m["l0"] = l0[s:s + SHARD]
        m["l1"] = l1[s:s + SHARD]
        in_maps.append(m)

    res = run_bass_kernel_spmd(nc, in_maps, core_ids=list(range(N_CORES)),
                               trace=trace, tmpdir=tmpdir)

    # reassemble per-atom v (dipole) and s (charge)
    v = np.empty((N_ATOMS, 3), dtype=np.float32)
    charge = np.empty((N_ATOMS,), dtype=np.float32)
    for c in range(N_CORES):
        o = res.results[c]["out"]  # [4, SHARD]
        lo = c * ATOMS_PER_CORE
        off = lo - starts[c]
        v[lo:lo + ATOMS_PER_CORE] = o[0:3, off:off + ATOMS_PER_CORE].T
        charge[lo:lo + ATOMS_PER_CORE] = o[3, off:off + ATOMS_PER_CORE]

    y_atom = v + pos * charge[:, None]
    y = np.stack(
        [np.bincount(batch_np, weights=y_atom[:, k].astype(np.float64),
                     minlength=nseg)[:nseg] for k in range(3)],
        axis=1).astype(np.float32)
    y_vec = np.stack(
        [np.bincount(batch_np, weights=v[:, k].astype(np.float64),
                     minlength=nseg)[:nseg] for k in range(3)],
        axis=1).astype(np.float32)[..., None]

    if trace:
        kernel.last_results = res
    return y, y_vec


kernel.last_results = None


# revision 27
# speedup vs baseline: 1.3294x; 1.0020x over previous
"""Trainium2 Bass kernel for nn_Dipole (gated equivariant MLP + segment reduce).

Strategy:
  - Shard 200000 atoms across 8 NeuronCores (overlapping 25088-atom windows,
    25088 = 49 * 512 so every core runs an identical fully-unrolled program).
  - On device (per core): cast-DMA l0/l1 to bf16, PE-transpose input tiles to
    feature-major layout, run both gated blocks as bf16 matmuls (weights
    replicated), emit per-atom dipole v [3] + charge s [1].
  - On host: y_atom = v + pos*charge, segment-sum per molecule via bincount.

Layouts are c-major over the full 512-atom block so every elementwise op is a
unit-stride [128, 512] op. vW matmuls are deferred until the gate is ready so
the gating multiply consumes vW directly from PSUM (no copy).

Self-contained: hardcodes shapes; only needs numpy + the concourse/axon stack
that ships with the container.
"""

import os
import sys

import numpy as np

for _p in ("/opt/trn_rl_repo", "/root/.axon_site/_ro/trn_rl_repo"):
    if os.path.isdir(_p) and _p not in sys.path:
        sys.path.append(_p)

import concourse.bacc as bacc
import concourse.bass as bass
import concourse.tile as tile
from concourse import mybir
from concourse.bass_utils import run_bass_kernel_spmd
from concourse.masks import make_identity

# run_bass_kernel_spmd(trace=True) imports antenv.axon_hooks, which this image
# lacks; register a working (or None) hook implementation up front so tracing
# degrades gracefully instead of crashing.
try:
    import antenv  # noqa
    if "antenv.axon_hooks" not in sys.modules:
        import types as _types

        def _mk_hook():
            try:
                from trn_agent_boot.trn_boot import _ntff_profile_via_ctypes
                return _ntff_profile_via_ctypes("/opt/axon/libaxon_pjrt.so")
            except Exception:
                return None

        _hook = _mk_hook()
        _mod = _types.ModuleType("antenv.axon_hooks")
        _mod.get_axon_ntff_profile_hook = lambda: _hook
        _mod.set_axon_ntff_profile_hook = lambda h: None
        sys.modules["antenv.axon_hooks"] = _mod
        antenv.axon_hooks = _mod
    import concourse.bass_utils as _bu

    def _local_upload(tmpdir):
        return str(tmpdir)

    _bu.upload_artifacts = _local_upload
except Exception:
    pass

F32 = mybir.dt.float32
BF16 = mybir.dt.bfloat16
AF = mybir.ActivationFunctionType
SILU = AF.Silu  # simcheck swaps to Tanh (CoreSim lacks Silu)

N_CORES = 8
N_ATOMS = 200000
N_IN = 256
N_HID = 256
N_MOL = 2048
BLK = 512                      # atoms per pipeline block
NQ = BLK // 128                # 128-atom quarters per block
SHARD = 25088                  # = 49 * BLK, per-core window
N_BLOCKS = SHARD // BLK
ATOMS_PER_CORE = N_ATOMS // N_CORES  # 25000 rows of output actually used


def bcast_c(ap2d, c=3):
    """[P, A] AP -> [P, c, A] with a step-0 broadcast middle dim."""
    return bass.AP(tensor=ap2d.tensor, offset=ap2d.offset,
                   ap=[ap2d.ap[0], [0, c], ap2d.ap[1]])


def build_nc(n_blocks=N_BLOCKS):
    n = n_blocks * BLK
    nc = bacc.Bacc("TRN2", target_bir_lowering=False, debug=False)

    l0 = nc.dram_tensor("l0", [n, N_IN], F32, kind="ExternalInput")
    l1 = nc.dram_tensor("l1", [n, 3, N_IN], F32, kind="ExternalInput")
    Wmix1 = nc.dram_tensor("Wmix1", [N_IN, 2 * N_HID], F32, kind="ExternalInput")
    W1a = nc.dram_tensor("W1a", [N_IN + N_HID, N_HID], F32, kind="ExternalInput")
    b1a = nc.dram_tensor("b1a", [N_HID], F32, kind="ExternalInput")
    W2a = nc.dram_tensor("W2a", [N_HID, 2 * N_HID], F32, kind="ExternalInput")
    b2a = nc.dram_tensor("b2a", [2 * N_HID], F32, kind="ExternalInput")
    Wmix2 = nc.dram_tensor("Wmix2", [N_HID, 2], F32, kind="ExternalInput")
    W1b = nc.dram_tensor("W1b", [N_HID + 1, N_HID], F32, kind="ExternalInput")
    b1b = nc.dram_tensor("b1b", [N_HID], F32, kind="ExternalInput")
    W2b = nc.dram_tensor("W2b", [N_HID, 2], F32, kind="ExternalInput")
    b2b = nc.dram_tensor("b2b", [2], F32, kind="ExternalInput")
    out = nc.dram_tensor("out", [4, n], F32, kind="ExternalOutput")

    with tile.TileContext(nc) as tc:
        with (
            tc.tile_pool(name="singles", bufs=1) as singles,
            tc.tile_pool(name="pin", bufs=5) as pin,
            tc.tile_pool(name="ptr", bufs=2) as ptr,
            tc.tile_pool(name="pact", bufs=2) as pact,
            tc.tile_pool(name="pmisc", bufs=3) as pmisc,
            tc.tile_pool(name="pout", bufs=3) as pout,
            tc.tile_pool(name="ps_tr", bufs=2, space="PSUM") as ps_tr,
            tc.tile_pool(name="ps_a", bufs=2, space="PSUM") as ps_a,
            tc.tile_pool(name="ps_b", bufs=2, space="PSUM") as ps_b,
            tc.tile_pool(name="ps_c", bufs=2, space="PSUM") as ps_c,
        ):
            # ---- constants / weights (bf16 via SWDGE cast-DMA) ----
            ident = singles.tile([128, 128], BF16)
            make_identity(nc, ident)

            wmix1_sb = singles.tile([128, 2, 512], BF16)
            nc.gpsimd.dma_start(
                out=wmix1_sb, in_=Wmix1.rearrange("(ko ki) o -> ki ko o", ki=128))
            w1a_sb = singles.tile([128, 4, 256], BF16)
            nc.gpsimd.dma_start(
                out=w1a_sb, in_=W1a.rearrange("(ko ki) o -> ki ko o", ki=128))
            w2a_sb = singles.tile([128, 2, 512], BF16)
            nc.gpsimd.dma_start(
                out=w2a_sb, in_=W2a.rearrange("(ko ki) o -> ki ko o", ki=128))
            w1b_sb = singles.tile([128, 2, 256], BF16)
            nc.gpsimd.dma_start(
                out=w1b_sb, in_=W1b[0:256, :].rearrange("(ko ki) o -> ki ko o", ki=128))
            w1bL_sb = singles.tile([1, 256], BF16)
            nc.gpsimd.dma_start(out=w1bL_sb, in_=W1b[256:257, :])
            w2b_sb = singles.tile([128, 2, 2], BF16)
            nc.gpsimd.dma_start(
                out=w2b_sb, in_=W2b.rearrange("(ko ki) o -> ki ko o", ki=128))
            wmix2_sb = singles.tile([128, 2, 2], BF16)
            nc.gpsimd.dma_start(
                out=wmix2_sb, in_=Wmix2.rearrange("(ko ki) o -> ki ko o", ki=128))

            b1a_sb = singles.tile([128, 2], F32)
            nc.sync.dma_start(out=b1a_sb, in_=b1a.rearrange("(mo mi) -> mi mo", mi=128))
            b2a_sb = singles.tile([128, 4], F32)
            nc.sync.dma_start(out=b2a_sb, in_=b2a.rearrange("(mo mi) -> mi mo", mi=128))
            b1b_sb = singles.tile([128, 2], F32)
            nc.sync.dma_start(out=b1b_sb, in_=b1b.rearrange("(mo mi) -> mi mo", mi=128))
            b2b_sb = singles.tile([2, 1], F32)
            nc.sync.dma_start(out=b2b_sb, in_=b2b.rearrange("(p o) -> p o", o=1))

            for b in range(n_blocks):
                a0 = b * BLK
                # ---- load + cast inputs (atom-major) ----
                l0_t = pin.tile([128, NQ, 256], BF16, tag="l0_t")
                nc.gpsimd.dma_start(
                    out=l0_t,
                    in_=l0[a0:a0 + BLK, :].rearrange("(ao ai) f -> ai ao f", ai=128))
                l1_t = pin.tile([128, NQ, 3, 256], BF16, tag="l1_t")
                nc.gpsimd.dma_start(
                    out=l1_t,
                    in_=l1[a0:a0 + BLK, :, :].rearrange("(ao ai) c f -> ai ao c f", ai=128))

                # ---- transpose to feature-major (quarter layout) ----
                # l0T: (ki, kt, q*128+a); l1T: (ki, kt, q, (c, a))
                l0T = ptr.tile([128, 2, BLK], BF16, tag="l0T")
                l1T = ptr.tile([128, 2, NQ, 384], BF16, tag="l1T")
                for qp in range(NQ // 2):
                    for kt in range(2):
                        trp = ps_tr.tile([128, 1024], BF16, tag="trp", name="trp")
                        for qi in range(2):
                            q = qp * 2 + qi
                            for c in range(3):
                                nc.tensor.transpose(
                                    trp[:, qi * 384 + c * 128:qi * 384 + (c + 1) * 128],
                                    l1_t[:, q, c, kt * 128:(kt + 1) * 128], ident)
                            nc.tensor.transpose(
                                trp[:, 768 + q % 2 * 128:768 + (q % 2 + 1) * 128],
                                l0_t[:, q, kt * 128:(kt + 1) * 128], ident)
                        nc.vector.tensor_copy(
                            l1T[:, kt, qp * 2:qp * 2 + 2, :], trp[:, 0:768])
                        nc.scalar.copy(
                            l0T[:, kt, qp * 256:(qp + 1) * 256], trp[:, 768:1024])

                # ---- block 1: vmix1; vVn; vW kept in SBUF ----
                vvnsq = pmisc.tile([128, 2, BLK], F32, tag="vvnsq")
                vvn_bf = pact.tile([128, 2, BLK], BF16, tag="vvn")
                vw_sb = pact.tile([128, 2, NQ, 384], BF16, tag="vw")
                for mt in range(4):
                    for qp in range(NQ // 2):
                        vps = [None, None]
                        for kt in range(2):          # weight reused across qi
                            for qi in range(2):
                                if kt == 0:
                                    vps[qi] = ps_a.tile([128, 384], F32, tag="vps", name="vps")
                                nc.tensor.matmul(
                                    vps[qi],
                                    wmix1_sb[:, kt, mt * 128:(mt + 1) * 128],
                                    l1T[:, kt, qp * 2 + qi, :],
                                    start=(kt == 0), stop=(kt == 1))
                        for qi in range(2):
                            q = qp * 2 + qi
                            if mt < 2:
                                sq = pmisc.tile([128, 384], F32, tag="sq", name="sq")
                                nc.scalar.square(sq, vps[qi])
                                nc.vector.tensor_reduce(
                                    vvnsq[:, mt, q * 128:(q + 1) * 128],
                                    sq[:, :].rearrange("p (c a) -> p a c", c=3),
                                    axis=mybir.AxisListType.X, op=mybir.AluOpType.add)
                            else:
                                nc.vector.tensor_copy(
                                    vw_sb[:, mt - 2, q, :], vps[qi])
                nc.scalar.sqrt(vvn_bf[:, :, :], vvnsq[:, :, :])

                # ---- block 1 MLP ----
                h1_bf = pact.tile([128, 2, BLK], BF16, tag="h1")
                for mt in range(2):
                    hps = ps_b.tile([128, BLK], F32, tag="hps", name="hps")
                    for kt in range(4):
                        rhs = l0T[:, kt, :] if kt < 2 else vvn_bf[:, kt - 2, :]
                        nc.tensor.matmul(
                            hps, w1a_sb[:, kt, mt * 128:(mt + 1) * 128], rhs,
                            start=(kt == 0), stop=(kt == 3))
                    nc.scalar.activation(
                        out=h1_bf[:, mt, :], in_=hps, func=SILU,
                        bias=b1a_sb[:, mt:mt + 1], scale=1.0)

                s_bf = pact.tile([128, 2, BLK], BF16, tag="s")
                gate_bf = pact.tile([128, 2, BLK], BF16, tag="gate")
                for mt in range(4):
                    xps = ps_b.tile([128, BLK], F32, tag="hps", name="xps")
                    for kt in range(2):
                        nc.tensor.matmul(
                            xps, w2a_sb[:, kt, mt * 128:(mt + 1) * 128], h1_bf[:, kt, :],
                            start=(kt == 0), stop=(kt == 1))
                    if mt < 2:
                        nc.scalar.activation(
                            out=s_bf[:, mt, :], in_=xps, func=SILU,
                            bias=b2a_sb[:, mt:mt + 1], scale=1.0)
                    else:
                        nc.scalar.activation(
                            out=gate_bf[:, mt - 2, :], in_=xps, func=AF.Identity,
                            bias=b2a_sb[:, mt:mt + 1], scale=1.0)

                # ---- gating: v_out = gate * vW (GPSIMD, SBUF only) ----
                vout_bf = pact.tile([128, 2, NQ, 384], BF16, tag="vout")
                for mt in range(2):
                    g = gate_bf[:, mt, :]
                    gb = bass.AP(tensor=g.tensor, offset=g.offset,
                                 ap=[g.ap[0], [128, NQ], [0, 3], [1, 128]])
                    nc.vector.tensor_mul(
                        vout_bf[:, mt, :, :].rearrange("p q (c a) -> p q c a", c=3),
                        vw_sb[:, mt, :, :].rearrange("p q (c a) -> p q c a", c=3),
                        gb)

                # ---- block 2: vmix2 (M=2 per c at 32-aligned strips) ----
                smp = ps_c.tile([128, BLK], F32, tag="csm", name="smp")
                for kt in range(2):                 # weight reused across c
                    for c in range(3):
                        nc.tensor.matmul(
                            smp[32 * c:32 * c + 2, :],
                            wmix2_sb[:, kt, :],
                            vout_bf[:, kt, :, c * 128:(c + 1) * 128],
                            start=(kt == 0), stop=(kt == 1))

                # vVn2 = sqrt(sum_c vV2^2); vV2 = rows 32c, vW2 = rows 32c+1.
                t_sq = pmisc.tile([2, 3, BLK], F32, tag="t_sq")
                for c in range(3):
                    nc.scalar.square(t_sq[:, c, :], smp[32 * c:32 * c + 2, :])
                vvn2sq = pmisc.tile([1, BLK], F32, tag="vvn2sq")
                nc.gpsimd.tensor_add(vvn2sq, t_sq[0:1, 0, :], t_sq[0:1, 1, :])
                nc.gpsimd.tensor_add(vvn2sq, vvn2sq, t_sq[0:1, 2, :])
                vvn2_bf = pmisc.tile([1, BLK], BF16, tag="vvn2bf")
                nc.scalar.sqrt(vvn2_bf, vvn2sq)

                # ---- block 2 MLP ----
                h2_bf = pact.tile([128, 2, BLK], BF16, tag="h2")
                for mt in range(2):
                    h2ps = ps_b.tile([128, BLK], F32, tag="hps", name="h2ps")
                    for kt in range(2):
                        nc.tensor.matmul(
                            h2ps, w1b_sb[:, kt, mt * 128:(mt + 1) * 128], s_bf[:, kt, :],
                            start=(kt == 0), stop=False)
                    nc.tensor.matmul(
                        h2ps, w1bL_sb[0:1, mt * 128:(mt + 1) * 128], vvn2_bf,
                        start=False, stop=True)
                    nc.scalar.activation(
                        out=h2_bf[:, mt, :], in_=h2ps, func=SILU,
                        bias=b1b_sb[:, mt:mt + 1], scale=1.0)

                x3ps = ps_c.tile([2, BLK], F32, tag="csm", name="x3ps")
                for kt in range(2):
                    nc.tensor.matmul(
                        x3ps, w2b_sb[:, kt, :], h2_bf[:, kt, :],
                        start=(kt == 0), stop=(kt == 1))
                x3_sb = pout.tile([2, BLK], F32, tag="x3sb")
                nc.scalar.activation(
                    out=x3_sb, in_=x3ps, func=AF.Identity,
                    bias=b2b_sb[0:2, 0:1], scale=1.0)

                # ---- v_final = gate2 * vW2 (row 1 of each [2,.] pair) ----
                vfin = pout.tile([2, 3, BLK], F32, tag="vfin")
                for c in range(3):
                    nc.vector.tensor_mul(
                        vfin[:, c, :], smp[32 * c:32 * c + 2, :], x3_sb[0:2, :])

                nc.sync.dma_start(out=out[0:3, a0:a0 + BLK], in_=vfin[1:2, :, :])
                nc.sync.dma_start(out=out[3:4, a0:a0 + BLK], in_=x3_sb[0:1, :])

    nc.compile()
    return nc


_NC_CACHE = {}


def _get_nc(n_blocks=N_BLOCKS, finalized=True):
    key = (n_blocks, finalized)
    if key not in _NC_CACHE:
        nc = build_nc(n_blocks)
        if finalized:
            nc.finalize()
        _NC_CACHE[key] = nc
    return _NC_CACHE[key]


def kernel(pos, l0, l1, batch, Wmix1, W1a, b1a, W2a, b2a,
           Wmix2, W1b, b1b, W2b, b2b, num_segments, trace=False, tmpdir=None):
    pos = np.asarray(pos, dtype=np.float32)
    l0 = np.asarray(l0, dtype=np.float32)
    l1 = np.asarray(l1, dtype=np.float32)
    batch_np = np.asarray(batch).astype(np.int64)
    nseg = int(num_segments)

    nc = _get_nc()

    weights = dict(Wmix1=Wmix1, W1a=W1a, b1a=b1a, W2a=W2a, b2a=b2a,
                   Wmix2=Wmix2, W1b=W1b, b1b=b1b, W2b=W2b, b2b=b2b)
    weights = {k: np.ascontiguousarray(np.asarray(v, dtype=np.float32))
               for k, v in weights.items()}

    starts = [min(c * ATOMS_PER_CORE, N_ATOMS - SHARD) for c in range(N_CORES)]
    in_maps = []
    for c in range(N_CORES):
        s = starts[c]
        m = dict(weights)
        m["l0"] = l0[s:s + SHARD]
        m["l1"] = l1[s:s + SHARD]
        in_maps.append(m)

    res = run_bass_kernel_spmd(nc, in_maps, core_ids=list(range(N_CORES)),
                               trace=trace, tmpdir=tmpdir)

    # reassemble per-atom v (dipole) and s (charge)
    v = np.empty((N_ATOMS, 3), dtype=np.float32)
    charge = np.empty((N_ATOMS,), dtype=np.float32)
    for c in range(N_CORES):
        o = res.results[c]["out"]  # [4, SHARD]
        lo = c * ATOMS_PER_CORE
        off = lo - starts[c]
        v[lo:lo + ATOMS_PER_CORE] = o[0:3, off:off + ATOMS_PER_CORE].T
        charge[lo:lo + ATOMS_PER_CORE] = o[3, off:off + ATOMS_PER_CORE]

    y_atom = v + pos * charge[:, None]
    y = np.stack(
        [np.bincount(batch_np, weights=y_atom[:, k].astype(np.float64),
                     minlength=nseg)[:nseg] for k in range(3)],
        axis=1).astype(np.float32)
    y_vec = np.stack(
        [np.bincount(batch_np, weights=v[:, k].astype(np.float64),
                     minlength=nseg)[:nseg] for k in range(3)],
        axis=1).astype(np.float32)[..., None]

    if trace:
        kernel.last_results = res
    return y, y_vec


kernel.last_results = None
